# revision 19
# baseline (speedup 1.0000x reference)
"""Trainium2 Bass kernel for additive (Bahdanau-style) attention scoring.

Computes, for hidden [B,H], encoder_outputs [B,S,H], W_attn [2H,H], b_attn [H], v [H]:
    energy    = tanh(hidden @ W1 + enc @ W2 + b_attn)   (per (b,s) row)
    attention = softmax_S(energy @ v)                   -> [B, S]

Sharding: data-parallel over batch across 8 NeuronCores (2 batches/core);
weights replicated.  Per-core compute is a 4096x1024x1024 GEMM + tanh +
v-dot + softmax, laid out as zT tiles [k=128 partitions, s free] so the
tanh bias is a per-partition AP on the scalar engine.

v4 structure (vs the 171.5us v2):
 - enc transposes moved off the PE entirely: one XBAR dma_start_transpose
   per (block, j-quarter) turns a [128,1024] nat slab into all 8 encT
   chunk columns (SBUF->SBUF, DMA engines).  This removes ~30us of
   LDWEIGHTS-bound PE transposes AND ~43us of DVE psum->SBUF copies, and
   frees 2 PSUM banks (psz ring grows to 6, fewer WAR stalls).
 - inputs packed to 4 tensors (enc / W / smalls / ident) to shrink the
   NEFF wrapper's per-tensor staging + semaphore-reset epilogue.
 - dummies write the pscb ring (dep-free until cbias), ident preloads on
   the sync queue.
 - tail: block 7 runs per-kc tanh interleaved with its GEMM emission and
   accumulates the v-dot as 8 PSUM matmuls (no stt chain); exp reads
   PSUM directly everywhere; per-batch sums land in one [1,4] tile
   combined by a single accumulating activation; one scale op + one
   output DMA per batch.
"""

import sys
import types

import numpy as np

B, S, H = 16, 2048, 1024
N_CORES = 8
B_LOC = B // N_CORES  # 2 batches per core
HC = H // 128         # 8 contraction chunks
KC = H // 128         # 8 output-feature chunks
RB = 512              # rows (s positions) per block
NRB = S // RB         # 4 r-blocks per batch
NBLK = B_LOC * NRB    # 8 blocks per core

N_DUM1 = 90           # warm-up matmuls from ident arrival
N_DUM2 = 60           # gap-filler matmuls until w2 half 0 lands


def _ensure_axon_hooks():
    """Register the NTFF profile hook if the image's antenv lacks it."""
    try:
        import antenv.axon_hooks  # noqa: F401
        return
    except ImportError:
        pass
    try:
        import antenv
        from trn_agent_boot.trn_boot import _ntff_profile_via_ctypes
    except ImportError:
        return
    mod = types.ModuleType("antenv.axon_hooks")
    _hook = [None]
    mod.set_axon_ntff_profile_hook = lambda h: _hook.__setitem__(0, h)
    mod.get_axon_ntff_profile_hook = lambda: _hook[0]
    antenv.axon_hooks = mod
    sys.modules["antenv.axon_hooks"] = mod
    try:
        hook = _ntff_profile_via_ctypes("/opt/axon/libaxon_pjrt.so")
        mod.set_axon_ntff_profile_hook(hook)
    except Exception:
        pass


_ensure_axon_hooks()

import concourse.bass as bass  # noqa: E402,F401
import concourse.mybir as mybir  # noqa: E402
import concourse.tile as tile  # noqa: E402
from concourse import bacc  # noqa: E402
from concourse.bass_utils import run_bass_kernel_spmd  # noqa: E402
from concourse.tile_rust import add_dep_helper  # noqa: E402

f32 = mybir.dt.float32
f32r = mybir.dt.float32r
f16 = mybir.dt.float16
AF = mybir.ActivationFunctionType
ALU = mybir.AluOpType


def build_kernel():
    nc = bacc.Bacc("TRN2", target_bir_lowering=False, debug=False,
                   num_devices=N_CORES)

    enc = nc.dram_tensor("enc", [B_LOC, S, H], f32, kind="ExternalInput")
    # w rows 0:H = W1, rows H:2H = W2 (as in the problem's W_attn)
    w = nc.dram_tensor("w", [2 * H, H], f32, kind="ExternalInput")
    # smalls = [hid(b0) | hid(b1) | b_attn | v], 4096 f32
    smalls = nc.dram_tensor("smalls", [4 * H], f32, kind="ExternalInput")
    identd = nc.dram_tensor("ident", [128, 128], f16, kind="ExternalInput")
    out = nc.dram_tensor("out", [B_LOC, S], f32, kind="ExternalOutput")

    with tile.TileContext(nc) as tc, \
         tc.tile_pool(name="weights", bufs=1) as wpool, \
         tc.tile_pool(name="consts", bufs=1) as cpool, \
         tc.tile_pool(name="nat", bufs=4) as natpool, \
         tc.tile_pool(name="encT", bufs=3) as tpool, \
         tc.tile_pool(name="energy", bufs=24) as epool, \
         tc.tile_pool(name="zhalf", bufs=8) as zapool, \
         tc.tile_pool(name="zspill", bufs=32) as zpool, \
         tc.tile_pool(name="zsum", bufs=8) as zspool, \
         tc.tile_pool(name="acc", bufs=8) as accpool, \
         tc.tile_pool(name="sm", bufs=1) as smpool, \
         tc.tile_pool(name="psz", bufs=6, space="PSUM") as pszpool, \
         tc.tile_pool(name="pscb", bufs=2, space="PSUM") as pscbpool:

        # ---- HWDGE load (sync queue, earliest body entry) ----------------
        ident = cpool.tile([128, 128], f16, tag="ident")
        nc.sync.dma_start(ident[:], identd.ap())

        # ---- SWDGE (gpsimd) loads: enc + W2 + small operands + W1 --------
        nat = {}

        def load_nat(i):
            b, rb = divmod(i, NRB)
            t = natpool.tile([128, (RB // 128) * H], f16, tag="nat")
            r0 = rb * RB
            nc.gpsimd.dma_start(
                t[:].rearrange("p (j h) -> p j h", h=H),
                enc[b, r0:r0 + RB, :].rearrange("(j p) h -> p j h", p=128))
            nat[i] = t

        w2sb = wpool.tile([128, HC * H], f16, tag="w2sb")
        w1sb = wpool.tile([128, HC * H], f16, tag="w1sb")

        def load_w2_half(dst, half):
            c0 = half * 4
            nc.gpsimd.dma_start(
                dst[:].rearrange("p (c k) -> p c k", k=H)[:, c0:c0 + 4, :],
                w[H + c0 * 128:H + (c0 + 4) * 128, :].rearrange(
                    "(c p) k -> p c k", p=128))

        def load_w1_khalf(dst, kh):
            k0 = kh * 512
            nc.gpsimd.dma_start(
                dst[:].rearrange("p (c k) -> p c k", k=H)[:, :, k0:k0 + 512],
                w[:H, k0:k0 + 512].rearrange("(c p) k -> p c k", p=128))

        def w2ap(hc, kc):
            return w2sb[:, hc * H + kc * 128: hc * H + (kc + 1) * 128]

        load_nat(0)
        load_w2_half(w2sb, 0)
        load_w2_half(w2sb, 1)
        load_nat(1)
        # small transposed operands (4-byte SWDGE packets, tiny):
        # hidT[p, c*B_LOC+b] = hid[b, c*128+p], f16 cast
        hidT = cpool.tile([128, HC * B_LOC], f16, tag="hidT")
        for b in range(B_LOC):
            nc.gpsimd.dma_start(
                hidT[:].rearrange("p (c b) -> p c b", b=B_LOC)[:, :, b],
                smalls[b * H:(b + 1) * H].rearrange("(c p) -> p c", p=128))
        # bvT[:, 0:8] = b_attn chunks, bvT[:, 8:16] = v chunks (f32)
        bvT = cpool.tile([128, 2 * KC], f32, tag="bvT")
        nc.gpsimd.dma_start(
            bvT[:].rearrange("p (t c) -> p t c", t=2),
            smalls[2 * H:4 * H].rearrange("(t c p) -> p t c", t=2, p=128))
        def battnT_col(kc):
            return bvT[:, kc:kc + 1]

        def vT_col(kc):
            return bvT[:, KC + kc:KC + kc + 1]
        load_nat(2)
        load_nat(3)
        load_w1_khalf(w1sb, 0)
        load_w1_khalf(w1sb, 1)
        for i in range(4, NBLK):
            load_nat(i)

        # ---- PE warm-up dummies -----------------------------------------
        # Keep HAM's clock gate open while the first DMAs land.  The
        # dummies only depend on ident and write junk to the pscb ring
        # (idle until cbias at ~55us), so they never couple with the
        # GEMM's psz ring.
        def dummies(n):
            for _ in range(n):
                pd = pscbpool.tile([128, 128], f32, tag="pscb")
                nc.tensor.matmul(pd[:], ident[:], ident[:],
                                 start=True, stop=True)

        dummies(N_DUM1)

        # ---- XBAR transposes (DMA engines, not PE) ----------------------
        # One dma_start_transpose per (block, j-quarter): nat slab
        # [128s, 1024h] -> 128-column stripes of all 8 encT chunks.  The
        # sync HWDGE queue paces them on each nat tile's arrival.
        encTs = {}

        def do_transposes(i):
            nt = nat[i]
            et = tpool.tile([128, HC * RB], f16, tag="encT")
            dst = et[:].rearrange("p (e jj c) -> p e jj c", e=HC,
                                  jj=RB // 128)
            for j in range(RB // 128):
                nc.sync.dma_start_transpose(
                    dst[:, :, j, :], nt[:, j * H:(j + 1) * H])
            encTs[i] = [et[:, hc * RB:(hc + 1) * RB] for hc in range(HC)]

        for i in range(NBLK):
            do_transposes(i)
        dummies(N_DUM2)

        # ---- GEMM + tanh + v-accumulate per block -----------------------
        # Every block spills psz -> SBUF via DVE/scalar, so the PE never
        # waits on tanh (which is gated on cbias/W1 for the early blocks).
        # Block 0 additionally splits the contraction into two 4-chunk
        # halves so its GEMM can start on W2's first half ('splitA' spills
        # f32, 'splitB' fuses the halves into a f16 zsum via DVE stt).
        zA = {}       # block 0 half-A spills, per kc (f16 SBUF)
        zsums = {}    # block 0 combined pre-tanh, per kc (f16 SBUF)
        zfull = {}    # (i, kc) -> f16 SBUF spill for blocks 1-3
        psz_of = {}   # (i, kc) -> psz tile for direct blocks 4-7

        def gemm_block(i, mode="spill", kcs=None, after_kc=None):
            for kc in (kcs if kcs is not None else range(KC)):
                psz = pszpool.tile([128, RB], f32, tag="psz")
                hcs = (range(0, 4) if mode == "splitA"
                       else range(4, 8) if mode == "splitB"
                       else range(HC))
                for n, hc in enumerate(hcs):
                    nc.tensor.matmul(
                        psz[:], w2ap(hc, kc), encTs[i][hc],
                        start=(n == 0), stop=(n == len(hcs) - 1))
                if mode == "splitA":
                    z = zapool.tile([128, RB], f16, tag="zA")
                    nc.vector.tensor_copy(z[:], psz[:])
                    zA[kc] = z
                elif mode == "splitB":
                    zs = zspool.tile([128, RB], f16, tag="zsum")
                    nc.vector.scalar_tensor_tensor(
                        zs[:], psz[:], 0.0, zA[kc][:],
                        op0=ALU.add, op1=ALU.add)
                    zsums[kc] = zs
                elif mode == "spill":
                    z = zpool.tile([128, RB], f16, tag="zfull")
                    if i in (1, 2):
                        nc.scalar.activation(z[:], psz[:], AF.Copy)
                    else:
                        nc.vector.tensor_copy(z[:], psz[:])
                    zfull[(i, kc)] = z
                else:
                    psz_of[(i, kc)] = psz
                if after_kc is not None:
                    after_kc(kc)

        enss = {}

        def tanh_kc(i, kc):
            b = i // NRB
            ens = enss.setdefault(i, {})
            en = epool.tile([128, RB], f16, tag="energy")
            if i == 0:
                src = zsums[kc][:]
            elif (i, kc) in zfull:
                src = zfull.pop((i, kc))[:]
            else:
                src = psz_of.pop((i, kc))[:]
            nc.scalar.activation(
                en[:], src, AF.Tanh,
                bias=cbiasT[:, kc * B_LOC + b: kc * B_LOC + b + 1])
            ens[kc] = en

        def tanh_half(i, half):
            for kc in range(half * 4, half * 4 + 4):
                tanh_kc(i, kc)

        ones = cpool.tile([128, 1], f16, tag="ones")
        nc.vector.memset(ones[:], 1.0)
        vT16 = cpool.tile([128, KC], f16, tag="vT16")
        accs = {}
        psas = {}

        def stt_block(i):
            acc = accpool.tile([128, RB], f16, tag="acc")
            ens = enss[i]
            nc.vector.tensor_scalar_mul(acc[:], ens[0][:], vT_col(0))
            for kc in range(1, KC):
                nc.vector.scalar_tensor_tensor(
                    acc[:], ens[kc][:], vT_col(kc), acc[:],
                    op0=ALU.mult, op1=ALU.add)
            accs[i] = acc

        def matvec(i):
            # ones-matvec reduces acc's 128 partitions into [1, RB] logits
            psa = pscbpool.tile([1, RB], f32, tag="pscb")
            nc.tensor.matmul(psa[:], ones[:], accs[i][:],
                             start=True, stop=True)
            psas[i] = psa

        def vmatvec(i):
            # direct v-dot: 8 accumulating [128,1]x[128,RB] matmuls; skips
            # the DVE stt chain entirely (used for the final block).
            psa = pscbpool.tile([1, RB], f32, tag="pscb")
            ens = enss[i]
            for kc in range(KC):
                nc.tensor.matmul(psa[:], vT16[:, kc:kc + 1], ens[kc][:],
                                 start=(kc == 0), stop=(kc == KC - 1))
            psas[i] = psa

        # ---- softmax plumbing (per 512-chunk, overlapped) ---------------
        expo = {}
        ssumT = {}
        for b in range(B_LOC):
            ex = smpool.tile([1, S], f32, tag=f"expo_{b}")
            st = smpool.tile([1, NRB], f32, tag=f"ssumT_{b}")
            expo[b] = ex
            ssumT[b] = st

        def exp_chunk(i):
            b, rb = divmod(i, NRB)
            nc.scalar.activation(expo[b][:, rb * RB:(rb + 1) * RB],
                                 psas[i][:], AF.Exp,
                                 accum_out=ssumT[b][:, rb:rb + 1])

        def finalize_batch(b):
            scr = smpool.tile([1, NRB], f32, tag=f"scr_{b}")
            tot = smpool.tile([1, 1], f32, tag=f"tot_{b}")
            nc.scalar.activation(scr[:], ssumT[b][:], AF.Identity,
                                 accum_out=tot[:])
            rec = smpool.tile([1, 1], f32, tag=f"rec_{b}")
            nc.vector.reciprocal(rec[:], tot[:])
            nc.vector.tensor_scalar_mul(expo[b][:], expo[b][:], rec[:])
            nc.sync.dma_start(out[b:b + 1, :], expo[b][:])

        # ---- cbias: cb[b,k] = hidden@W1 + b_attn, transposed ------------
        # Emitted into the PE stream after G2 so W1's SWDGE arrival
        # (~50us) is off the critical path.
        cbiasT = cpool.tile([128, KC * B_LOC], f32, tag="cbiasT")
        cb16 = cpool.tile([B_LOC, H], f16, tag="cb16")

        def cbias_part(kh):
            # k-half of cb = hidden@W1 + b: gated only on W1's kh-th
            # k-half load, so the tanh conveyor can start on kc 0-3
            # while W1's second half is still in the SWDGE pipe.
            psc = pscbpool.tile([B_LOC, 512], f32, tag="pscb")
            for hc in range(HC):
                nc.tensor.matmul(
                    psc[:],
                    hidT[:, hc * B_LOC:(hc + 1) * B_LOC],
                    w1sb[:, hc * H + kh * 512: hc * H + (kh + 1) * 512],
                    start=(hc == 0), stop=(hc == HC - 1))
            nc.vector.tensor_copy(
                cb16[:, kh * 512:(kh + 1) * 512], psc[:])
            for kc in range(kh * 4, kh * 4 + 4):
                pt2 = pscbpool.tile([128, B_LOC], f16, tag="pscb")
                nc.tensor.transpose(
                    pt2[:], cb16[:, kc * 128:(kc + 1) * 128],
                    ident[0:B_LOC, 0:B_LOC])
                nc.scalar.activation(
                    cbiasT[:, kc * B_LOC:(kc + 1) * B_LOC], pt2[:],
                    AF.Identity, bias=battnT_col(kc))

        # ---- PE program ------------------------------------------------
        # [dum1, T0, dum2, GA0, GB0, T1, G1, T2, G2, cbias, T3, G3, T4,
        #  G4, mv0, T5, G5, mv1, mv2, T6, G6, mv3, mv4, T7, G7a, G7b
        #  (tanh7 per-kc inline), mv5, mv6, vmv7, mv..];
        # tanh/stt/exp ride the scalar/DVE queues, emitted so no gated op
        # sits ahead of a PE-feeding copy in an in-order queue.
        def tanh_stt(i):
            tanh_half(i, 0)
            tanh_half(i, 1)
            stt_block(i)

        gemm_block(0, "splitA")
        gemm_block(0, "splitB")
        gemm_block(1)
        gemm_block(2)
        cbias_part(0)
        gemm_block(3)
        tanh_half(0, 0)
        tanh_half(1, 0)
        tanh_half(2, 0)
        tanh_half(3, 0)
        cbias_part(1)
        nc.vector.tensor_copy(vT16[:], bvT[:, KC:2 * KC])
        tanh_half(0, 1)
        stt_block(0)
        tanh_half(1, 1)
        stt_block(1)
        gemm_block(4, "direct")
        tanh_half(2, 1)
        stt_block(2)
        tanh_half(3, 1)
        stt_block(3)
        gemm_block(5, "direct")
        tanh_stt(4)
        matvec(0)
        gemm_block(6, "direct")
        tanh_stt(5)
        matvec(1)
        exp_chunk(0)
        matvec(2)
        exp_chunk(1)
        exp_chunk(2)
        matvec(3)
        exp_chunk(3)
        gemm_block(7, "direct", kcs=range(0, 4))
        tanh_stt(6)
        matvec(4)
        exp_chunk(4)
        finalize_batch(0)
        matvec(5)
        exp_chunk(5)
        tanh_half(7, 0)
        # block 7's second GEMM half: tanh follows each kc immediately on
        # the scalar queue, so after the last matmul only ONE tanh remains
        # on the critical path.
        gemm_block(7, "direct", kcs=range(4, 8),
                   after_kc=lambda kc: tanh_kc(7, kc))
        matvec(6)
        vmatvec(7)
        exp_chunk(6)
        exp_chunk(7)
        finalize_batch(1)

    nc.compile()
    return nc


_NC_CACHE = None


def _get_nc():
    global _NC_CACHE
    if _NC_CACHE is None:
        _NC_CACHE = build_kernel()
    return _NC_CACHE


def kernel(hidden, encoder_outputs, W_attn, b_attn, v, _trace=False,
           _tmpdir=None):
    hidden = np.ascontiguousarray(hidden, dtype=np.float32)
    encoder_outputs = np.ascontiguousarray(encoder_outputs, dtype=np.float32)
    W_attn = np.ascontiguousarray(W_attn, dtype=np.float32)
    b_attn = np.ascontiguousarray(b_attn, dtype=np.float32)
    v = np.ascontiguousarray(v, dtype=np.float32)
    ident = np.eye(128, dtype=np.float16)

    nc = _get_nc()
    in_maps = []
    for c in range(N_CORES):
        b0 = c * B_LOC
        smalls = np.concatenate(
            [hidden[b0:b0 + B_LOC].reshape(-1), b_attn, v]).astype(np.float32)
        in_maps.append({
            "enc": encoder_outputs[b0:b0 + B_LOC],
            "w": W_attn,
            "smalls": smalls,
            "ident": ident,
        })
    res = run_bass_kernel_spmd(
        nc, in_maps, core_ids=list(range(N_CORES)),
        trace=_trace, tmpdir=_tmpdir)
    out = np.concatenate([res.results[c]["out"] for c in range(N_CORES)],
                         axis=0).astype(np.float32)
    if _trace:
        kernel.last_exec_time_ns = res.exec_time_ns
        kernel.last_results = res
    return out


# revision 20
# speedup vs baseline: 1.2752x; 1.2752x over previous
"""Trainium2 Bass kernel for additive (Bahdanau-style) attention scoring.

Computes, for hidden [B,H], encoder_outputs [B,S,H], W_attn [2H,H], b_attn [H], v [H]:
    energy    = tanh(hidden @ W1 + enc @ W2 + b_attn)   (per (b,s) row)
    attention = softmax_S(energy @ v)                   -> [B, S]

Sharding: data-parallel over batch across 8 NeuronCores (2 batches/core);
weights replicated.  Per-core compute is a 4096x1024x1024 GEMM + tanh +
v-dot + softmax, laid out as zT tiles [k=128 partitions, s free] so the
tanh bias is a per-partition AP on the scalar engine.

v5 structure (vs the 171.5us v2):
 - inputs packed to 4 tensors (enc / W / smalls / ident); ident preloads
   on the sync queue (enters the body earliest), dummies write the pscb
   ring (idle until cbias) so they never couple with the GEMM's psz ring.
 - tail: block 7 runs per-kc tanh interleaved with its GEMM emission and
   accumulates the v-dot as 8 PSUM matmuls (no stt chain); exp reads the
   matvec PSUM directly everywhere (no logit copies); per-batch sums land
   in one [1,4] tile combined by a single accumulating activation; one
   scale op + one output DMA per batch.
 - (v4's XBAR dma_start_transpose experiment was correct but 60us slower:
   it is lowered to 256B-packet descriptor storms on the same DMA engines
   as the loads.  PE transposes stay.)
"""

import sys
import types

import numpy as np

B, S, H = 16, 2048, 1024
N_CORES = 8
B_LOC = B // N_CORES  # 2 batches per core
HC = H // 128         # 8 contraction chunks
KC = H // 128         # 8 output-feature chunks
RB = 512              # rows (s positions) per block
NRB = S // RB         # 4 r-blocks per batch
NBLK = B_LOC * NRB    # 8 blocks per core

N_DUM1 = 78           # warm-up matmuls before first transposes
N_DUM2 = 70           # gap-filler matmuls between T(0) and GEMMh0


def _ensure_axon_hooks():
    """Register the NTFF profile hook if the image's antenv lacks it."""
    try:
        import antenv.axon_hooks  # noqa: F401
        return
    except ImportError:
        pass
    try:
        import antenv
        from trn_agent_boot.trn_boot import _ntff_profile_via_ctypes
    except ImportError:
        return
    mod = types.ModuleType("antenv.axon_hooks")
    _hook = [None]
    mod.set_axon_ntff_profile_hook = lambda h: _hook.__setitem__(0, h)
    mod.get_axon_ntff_profile_hook = lambda: _hook[0]
    antenv.axon_hooks = mod
    sys.modules["antenv.axon_hooks"] = mod
    try:
        hook = _ntff_profile_via_ctypes("/opt/axon/libaxon_pjrt.so")
        mod.set_axon_ntff_profile_hook(hook)
    except Exception:
        pass


_ensure_axon_hooks()

import concourse.bass as bass  # noqa: E402,F401
import concourse.mybir as mybir  # noqa: E402
import concourse.tile as tile  # noqa: E402
from concourse import bacc  # noqa: E402
from concourse.bass_utils import run_bass_kernel_spmd  # noqa: E402
from concourse.tile_rust import add_dep_helper  # noqa: E402

f32 = mybir.dt.float32
f32r = mybir.dt.float32r
f16 = mybir.dt.float16
AF = mybir.ActivationFunctionType
ALU = mybir.AluOpType


def build_kernel():
    nc = bacc.Bacc("TRN2", target_bir_lowering=False, debug=False,
                   num_devices=N_CORES)

    enc = nc.dram_tensor("enc", [B_LOC, S, H], f32, kind="ExternalInput")
    # w rows 0:H = W1, rows H:2H = W2 (as in the problem's W_attn)
    w = nc.dram_tensor("w", [2 * H, H], f32, kind="ExternalInput")
    # smalls = [hid(b0) | hid(b1) | b_attn | v], 4096 f32
    smalls = nc.dram_tensor("smalls", [4 * H], f32, kind="ExternalInput")
    identd = nc.dram_tensor("ident", [128, 128], f16, kind="ExternalInput")
    out = nc.dram_tensor("out", [B_LOC, S], f32, kind="ExternalOutput")

    with tile.TileContext(nc) as tc, \
         tc.tile_pool(name="weights", bufs=1) as wpool, \
         tc.tile_pool(name="consts", bufs=1) as cpool, \
         tc.tile_pool(name="nat", bufs=4) as natpool, \
         tc.tile_pool(name="encT", bufs=16) as tpool, \
         tc.tile_pool(name="energy", bufs=24) as epool, \
         tc.tile_pool(name="zhalf", bufs=8) as zapool, \
         tc.tile_pool(name="zspill", bufs=32) as zpool, \
         tc.tile_pool(name="zsum", bufs=8) as zspool, \
         tc.tile_pool(name="acc", bufs=8) as accpool, \
         tc.tile_pool(name="sm", bufs=1) as smpool, \
         tc.tile_pool(name="psz", bufs=4, space="PSUM") as pszpool, \
         tc.tile_pool(name="pst", bufs=2, space="PSUM") as pstpool, \
         tc.tile_pool(name="pscb", bufs=2, space="PSUM") as pscbpool:

        # ---- HWDGE load (sync queue, earliest body entry) ----------------
        ident = cpool.tile([128, 128], f16, tag="ident")
        nc.sync.dma_start(ident[:], identd.ap())

        # ---- SWDGE (gpsimd) loads: enc + W2 + small operands + W1 --------
        nat = {}

        def load_nat(i):
            b, rb = divmod(i, NRB)
            t = natpool.tile([128, (RB // 128) * H], f16, tag="nat")
            r0 = rb * RB
            nc.gpsimd.dma_start(
                t[:].rearrange("p (j h) -> p j h", h=H),
                enc[b, r0:r0 + RB, :].rearrange("(j p) h -> p j h", p=128))
            nat[i] = t

        w2sb = wpool.tile([128, HC * H], f16, tag="w2sb")
        w1sb = wpool.tile([128, HC * H], f16, tag="w1sb")

        def load_w2_half(dst, half):
            c0 = half * 4
            nc.gpsimd.dma_start(
                dst[:].rearrange("p (c k) -> p c k", k=H)[:, c0:c0 + 4, :],
                w[H + c0 * 128:H + (c0 + 4) * 128, :].rearrange(
                    "(c p) k -> p c k", p=128))

        def load_w1_khalf(dst, kh):
            k0 = kh * 512
            nc.gpsimd.dma_start(
                dst[:].rearrange("p (c k) -> p c k", k=H)[:, :, k0:k0 + 512],
                w[:H, k0:k0 + 512].rearrange("(c p) k -> p c k", p=128))

        def w2ap(hc, kc):
            return w2sb[:, hc * H + kc * 128: hc * H + (kc + 1) * 128]

        load_nat(0)
        load_w2_half(w2sb, 0)
        load_w2_half(w2sb, 1)
        load_nat(1)
        hidT = cpool.tile([128, HC * B_LOC], f16, tag="hidT")
        for b in range(B_LOC):
            nc.gpsimd.dma_start(
                hidT[:].rearrange("p (c b) -> p c b", b=B_LOC)[:, :, b],
                smalls[b * H:(b + 1) * H].rearrange("(c p) -> p c", p=128))
        # bvT[:, 0:8] = b_attn chunks, bvT[:, 8:16] = v chunks (f32)
        bvT = cpool.tile([128, 2 * KC], f32, tag="bvT")
        nc.gpsimd.dma_start(
            bvT[:].rearrange("p (t c) -> p t c", t=2),
            smalls[2 * H:4 * H].rearrange("(t c p) -> p t c", t=2, p=128))

        def battnT_col(kc):
            return bvT[:, kc:kc + 1]

        def vT_col(kc):
            return bvT[:, KC + kc:KC + kc + 1]

        load_nat(2)
        load_nat(3)
        load_w1_khalf(w1sb, 0)
        load_w1_khalf(w1sb, 1)
        for i in range(4, NBLK):
            load_nat(i)

        # ---- PE warm-up dummies -----------------------------------------
        # Keep HAM's clock gate open while the first DMAs land.  The
        # dummies only depend on ident and write junk to the pscb ring
        # (idle until cbias at ~55us), so they never couple with the
        # GEMM's psz ring.
        anchor = [None]   # last GEMM matmul of the most recent GEMM phase

        def dummies(n):
            for _ in range(n):
                pd = pscbpool.tile([128, 128], f32, tag="pscb")
                nc.tensor.matmul(pd[:], ident[:], ident[:],
                                 start=True, stop=True)

        dummies(N_DUM1)

        # ---- transposes -------------------------------------------------
        encTs = {}

        def do_transposes(i):
            # Anchor each transpose after the previous GEMM phase's last
            # matmul (same-engine ordering edge, no semaphore) so the PE
            # alternates pure-matmul and pure-transpose phases.
            nt = nat[i]
            tiles = []
            for hc in range(HC):
                tt = tpool.tile([128, RB], f16, tag="encT")
                pt = pstpool.tile([128, RB], f16, tag="pst")
                for j in range(RB // 128):
                    tr = nc.tensor.transpose(
                        pt[:, j * 128:(j + 1) * 128],
                        nt[:, j * H + hc * 128: j * H + (hc + 1) * 128],
                        ident[:])
                    if anchor[0] is not None:
                        add_dep_helper(anchor[0].ins, tr.ins, sync=False,
                                       reason="pe phase")
                nc.vector.tensor_copy(tt[:], pt[:])
                tiles.append(tt)
            encTs[i] = tiles

        do_transposes(0)
        dummies(N_DUM2)

        # ---- GEMM + tanh + v-accumulate per block -----------------------
        # Blocks 0-3 spill psz -> SBUF (DVE or scalar), so the PE never
        # waits on tanh (which is gated on cbias/W1 for the early blocks).
        # Block 0 additionally splits the contraction into two 4-chunk
        # halves so its GEMM can start on W2's first half.
        zA = {}       # block 0 half-A spills, per kc (f16 SBUF)
        zsums = {}    # block 0 combined pre-tanh, per kc (f16 SBUF)
        zfull = {}    # (i, kc) -> f16 SBUF spill for blocks 1-3
        psz_of = {}   # (i, kc) -> psz tile for direct blocks 4-7

        def gemm_block(i, mode="spill", kcs=None, after_kc=None):
            for kc in (kcs if kcs is not None else range(KC)):
                psz = pszpool.tile([128, RB], f32, tag="psz")
                hcs = (range(0, 4) if mode == "splitA"
                       else range(4, 8) if mode == "splitB"
                       else range(HC))
                for n, hc in enumerate(hcs):
                    mm = nc.tensor.matmul(
                        psz[:], w2ap(hc, kc), encTs[i][hc][:],
                        start=(n == 0), stop=(n == len(hcs) - 1))
                    anchor[0] = mm
                if mode == "splitA":
                    z = zapool.tile([128, RB], f16, tag="zA")
                    nc.vector.tensor_copy(z[:], psz[:])
                    zA[kc] = z
                elif mode == "splitB":
                    zs = zspool.tile([128, RB], f16, tag="zsum")
                    nc.vector.scalar_tensor_tensor(
                        zs[:], psz[:], 0.0, zA[kc][:],
                        op0=ALU.add, op1=ALU.add)
                    zsums[kc] = zs
                elif mode == "spill":
                    z = zpool.tile([128, RB], f16, tag="zfull")
                    if i in (1, 2):
                        nc.scalar.activation(z[:], psz[:], AF.Copy)
                    else:
                        nc.vector.tensor_copy(z[:], psz[:])
                    zfull[(i, kc)] = z
                else:
                    psz_of[(i, kc)] = psz
                if after_kc is not None:
                    after_kc(kc)

        enss = {}

        def tanh_kc(i, kc):
            b = i // NRB
            ens = enss.setdefault(i, {})
            en = epool.tile([128, RB], f16, tag="energy")
            if i == 0:
                src = zsums[kc][:]
            elif (i, kc) in zfull:
                src = zfull.pop((i, kc))[:]
            else:
                src = psz_of.pop((i, kc))[:]
            nc.scalar.activation(
                en[:], src, AF.Tanh,
                bias=cbiasT[:, kc * B_LOC + b: kc * B_LOC + b + 1])
            ens[kc] = en

        def tanh_half(i, half):
            for kc in range(half * 4, half * 4 + 4):
                tanh_kc(i, kc)

        ones = cpool.tile([128, 1], f16, tag="ones")
        nc.vector.memset(ones[:], 1.0)
        vT16 = cpool.tile([128, KC], f16, tag="vT16")
        accs = {}
        psas = {}

        def stt_block(i):
            acc = accpool.tile([128, RB], f16, tag="acc")
            ens = enss[i]
            nc.vector.tensor_scalar_mul(acc[:], ens[0][:], vT_col(0))
            for kc in range(1, KC):
                nc.vector.scalar_tensor_tensor(
                    acc[:], ens[kc][:], vT_col(kc), acc[:],
                    op0=ALU.mult, op1=ALU.add)
            accs[i] = acc

        def matvec(i):
            # ones-matvec reduces acc's 128 partitions into [1, RB] logits
            psa = pscbpool.tile([1, RB], f32, tag="pscb")
            nc.tensor.matmul(psa[:], ones[:], accs[i][:],
                             start=True, stop=True)
            psas[i] = psa

        def vmatvec(i):
            # direct v-dot: 8 accumulating [128,1]x[128,RB] matmuls; skips
            # the DVE stt chain entirely (used for the final block).
            psa = pscbpool.tile([1, RB], f32, tag="pscb")
            ens = enss[i]
            for kc in range(KC):
                nc.tensor.matmul(psa[:], vT16[:, kc:kc + 1], ens[kc][:],
                                 start=(kc == 0), stop=(kc == KC - 1))
            psas[i] = psa

        # ---- softmax plumbing (per 512-chunk, overlapped) ---------------
        expo = {}
        ssumT = {}
        for b in range(B_LOC):
            ex = smpool.tile([1, S], f32, tag=f"expo_{b}")
            st = smpool.tile([1, NRB], f32, tag=f"ssumT_{b}")
            expo[b] = ex
            ssumT[b] = st

        def exp_chunk(i):
            b, rb = divmod(i, NRB)
            nc.scalar.activation(expo[b][:, rb * RB:(rb + 1) * RB],
                                 psas[i][:], AF.Exp,
                                 accum_out=ssumT[b][:, rb:rb + 1])

        def finalize_batch(b):
            scr = smpool.tile([1, NRB], f32, tag=f"scr_{b}")
            tot = smpool.tile([1, 1], f32, tag=f"tot_{b}")
            nc.scalar.activation(scr[:], ssumT[b][:], AF.Identity,
                                 accum_out=tot[:])
            rec = smpool.tile([1, 1], f32, tag=f"rec_{b}")
            nc.vector.reciprocal(rec[:], tot[:])
            nc.vector.tensor_scalar_mul(expo[b][:], expo[b][:], rec[:])
            nc.sync.dma_start(out[b:b + 1, :], expo[b][:])

        # ---- cbias: cb[b,k] = hidden@W1 + b_attn, transposed ------------
        # Emitted into the PE stream after G2 so W1's SWDGE arrival
        # (~50us) is off the critical path.
        cbiasT = cpool.tile([128, KC * B_LOC], f32, tag="cbiasT")
        cb16 = cpool.tile([B_LOC, H], f16, tag="cb16")

        def cbias_part(kh):
            # k-half of cb = hidden@W1 + b: gated only on W1's kh-th
            # k-half load, so the tanh conveyor can start on kc 0-3
            # while W1's second half is still in the SWDGE pipe.
            psc = pscbpool.tile([B_LOC, 512], f32, tag="pscb")
            for hc in range(HC):
                nc.tensor.matmul(
                    psc[:],
                    hidT[:, hc * B_LOC:(hc + 1) * B_LOC],
                    w1sb[:, hc * H + kh * 512: hc * H + (kh + 1) * 512],
                    start=(hc == 0), stop=(hc == HC - 1))
            nc.vector.tensor_copy(
                cb16[:, kh * 512:(kh + 1) * 512], psc[:])
            for kc in range(kh * 4, kh * 4 + 4):
                pt2 = pscbpool.tile([128, B_LOC], f16, tag="pscb")
                nc.tensor.transpose(
                    pt2[:], cb16[:, kc * 128:(kc + 1) * 128],
                    ident[0:B_LOC, 0:B_LOC])
                nc.scalar.activation(
                    cbiasT[:, kc * B_LOC:(kc + 1) * B_LOC], pt2[:],
                    AF.Identity, bias=battnT_col(kc))

        # ---- PE program ------------------------------------------------
        # [dum1, T0, dum2, GA0, GB0, T1, G1, T2, G2, cbias, T3, G3, T4,
        #  G4, mv0, T5, G5, mv1, mv2, T6, G6, mv3, mv4, T7, G7a, G7b
        #  (tanh7 per-kc inline), mv5, mv6, vmv7];
        # tanh/stt/exp ride the scalar/DVE queues, emitted so no gated op
        # sits ahead of a PE-feeding copy in an in-order queue.
        def tanh_stt(i):
            tanh_half(i, 0)
            tanh_half(i, 1)
            stt_block(i)

        gemm_block(0, "splitA")
        gemm_block(0, "splitB")
        do_transposes(1)
        gemm_block(1, kcs=range(0, 2))
        do_transposes(2)
        gemm_block(1, kcs=range(2, 8))
        gemm_block(2, kcs=range(0, 2))
        do_transposes(3)
        gemm_block(2, kcs=range(2, 8))
        cbias_part(0)
        gemm_block(3, kcs=range(0, 2))
        do_transposes(4)
        gemm_block(3, kcs=range(2, 8))
        tanh_half(0, 0)
        tanh_half(1, 0)
        tanh_half(2, 0)
        tanh_half(3, 0)
        cbias_part(1)
        nc.vector.tensor_copy(vT16[:], bvT[:, KC:2 * KC])
        tanh_half(0, 1)
        stt_block(0)
        tanh_half(1, 1)
        stt_block(1)
        gemm_block(4, "direct", kcs=range(0, 2))
        do_transposes(5)
        gemm_block(4, "direct", kcs=range(2, 8))
        tanh_half(2, 1)
        stt_block(2)
        tanh_half(3, 1)
        stt_block(3)
        gemm_block(5, "direct", kcs=range(0, 2))
        do_transposes(6)
        gemm_block(5, "direct", kcs=range(2, 8))
        tanh_stt(4)
        matvec(0)
        gemm_block(6, "direct", kcs=range(0, 2))
        do_transposes(7)
        gemm_block(6, "direct", kcs=range(2, 8))
        tanh_stt(5)
        matvec(1)
        exp_chunk(0)
        matvec(2)
        exp_chunk(1)
        exp_chunk(2)
        matvec(3)
        exp_chunk(3)
        gemm_block(7, "direct", kcs=range(0, 4))
        tanh_stt(6)
        matvec(4)
        exp_chunk(4)
        finalize_batch(0)
        matvec(5)
        exp_chunk(5)
        tanh_half(7, 0)
        # block 7's second GEMM half: tanh follows each kc immediately on
        # the scalar queue, so after the last matmul only ONE tanh remains
        # on the critical path.
        gemm_block(7, "direct", kcs=range(4, 8),
                   after_kc=lambda kc: tanh_kc(7, kc))
        matvec(6)
        vmatvec(7)
        exp_chunk(6)
        exp_chunk(7)
        finalize_batch(1)

    nc.compile()
    return nc


_NC_CACHE = None


def _get_nc():
    global _NC_CACHE
    if _NC_CACHE is None:
        _NC_CACHE = build_kernel()
    return _NC_CACHE


def kernel(hidden, encoder_outputs, W_attn, b_attn, v, _trace=False,
           _tmpdir=None):
    hidden = np.ascontiguousarray(hidden, dtype=np.float32)
    encoder_outputs = np.ascontiguousarray(encoder_outputs, dtype=np.float32)
    W_attn = np.ascontiguousarray(W_attn, dtype=np.float32)
    b_attn = np.ascontiguousarray(b_attn, dtype=np.float32)
    v = np.ascontiguousarray(v, dtype=np.float32)
    ident = np.eye(128, dtype=np.float16)

    nc = _get_nc()
    in_maps = []
    for c in range(N_CORES):
        b0 = c * B_LOC
        smalls = np.concatenate(
            [hidden[b0:b0 + B_LOC].reshape(-1), b_attn, v]).astype(np.float32)
        in_maps.append({
            "enc": encoder_outputs[b0:b0 + B_LOC],
            "w": W_attn,
            "smalls": smalls,
            "ident": ident,
        })
    res = run_bass_kernel_spmd(
        nc, in_maps, core_ids=list(range(N_CORES)),
        trace=_trace, tmpdir=_tmpdir)
    out = np.concatenate([res.results[c]["out"] for c in range(N_CORES)],
                         axis=0).astype(np.float32)
    if _trace:
        kernel.last_exec_time_ns = res.exec_time_ns
        kernel.last_results = res
    return out
